# revision 5
# baseline (speedup 1.0000x reference)
"""Causal self-attention (B=2, T=2048, C=1024, 16 heads x 64) on 8 TRN2 NeuronCores.

Sharding: core c -> batch b = c//4, heads [4*(c%4), 4*(c%4)+4)  (data parallel on B,
tensor parallel on heads). Each core computes QKV for its 4 heads, causal attention,
and a partial output projection (its 256 columns of y against W_proj). Host sums the
4 partials per batch and adds b_proj.

All device layouts are pre-transposed on the host so the kernel needs no on-device
transposes:
  xT  (C[+1], T)  : x[b].T (optionally augmented with a ones row to fold b_qkv in)
  wq/wk/wv (C[+1], 256): W_qkv head-slices, transposed (+ bias row when augmented)
  wp  (256, C)    : W_proj[:, head_cols].T
  mk  (128, 512)  : causal mask tile, mk[s,u] = 1.0 if s <= u else 0.0

Device pipeline per core (all matmuls in float32r: fp32 storage, 1 cycle/row):
  qT[jt] (128,T): qT = Wq^T.T @ xT   (j on partitions: 2 tiles = head pairs)
  kT[jt] (128,T): same for K
  V (128,16,2,193): v in (s-part, d) layout per 128-row s-chunk, with a ones column
     fused per head so the P@V matmul also produces the softmax denominator:
     even head block = [v(64) | 1], odd head block = [1 | 0*63 | v(64)]
  sT = kT_chunk.T @ qT_chunk (s' on partitions, t free)  -- causal tiles only
  PT = exp(scale*sT) * mask  (ACT exp from PSUM, DVE mask mult on diagonal tiles)
  yT[h] += V_block.T @ PT    (accumulates y^T rows and the denominator row)
  yTn = yT * bcast(1/denom)  (reciprocal on DVE; broadcast over partitions via a
                              K=1 PE matmul with a ones vector)
  out_partial = yTn.T @ wp   (16 row-tiles of (128,1024))
"""

import numpy as np

import concourse.bass as bass
import concourse.mybir as mybir
import concourse.tile as tile
from concourse import bacc
from concourse.bass_utils import run_bass_kernel_spmd

F32R = mybir.dt.float32r
F32 = mybir.dt.float32
EXP = mybir.ActivationFunctionType.Exp

B, T, C = 2, 2048, 1024
N_HEAD, HD = 16, 64
NHL = 4                 # heads per core
JD = NHL * HD           # 256 local q/k/v dims per core
TT = T // 128           # 16 row tiles
TC4 = T // 512          # 4 column chunks
SCALE = 1.0 / 8.0       # 1/sqrt(64)
N_CORES = 8


def build_program(qkv_bias: bool):
    KQ = C + 1 if qkv_bias else C       # qkv contraction (+1 row folds b_qkv)
    KCQ = (KQ + 127) // 128             # k-chunks (last may be 1 row)

    nc = bacc.Bacc("TRN2", target_bir_lowering=False, debug=False)
    xT_d = nc.dram_tensor("xT", (KQ, T), F32R, kind="ExternalInput")
    wq_d = nc.dram_tensor("wq", (KQ, JD), F32R, kind="ExternalInput")
    wk_d = nc.dram_tensor("wk", (KQ, JD), F32R, kind="ExternalInput")
    wv_d = nc.dram_tensor("wv", (KQ, JD), F32R, kind="ExternalInput")
    wp_d = nc.dram_tensor("wp", (JD, C), F32R, kind="ExternalInput")
    mk_d = nc.dram_tensor("mk", (128, 512), F32R, kind="ExternalInput")
    out_d = nc.dram_tensor("out", (T, C), F32, kind="ExternalOutput")

    with tile.TileContext(nc) as tc, tc.tile_pool(name="persist", bufs=1) as pp:
        with (
            tc.tile_pool(name="xw", bufs=1) as xw,
            tc.tile_pool(name="psA", bufs=2, space="PSUM") as psA,
        ):
            qT = [pp.tile([128, T], F32R, tag=f"qT{j}", name=f"qT{j}") for j in range(2)]
            kT = [pp.tile([128, T], F32R, tag=f"kT{j}", name=f"kT{j}") for j in range(2)]
            V = pp.tile([128, TT, 2, 193], F32R, tag="V")
            yTn = [pp.tile([128, T], F32R, tag=f"yTn{j}", name=f"yTn{j}") for j in range(2)]
            ones = pp.tile([128, 128], F32R, tag="ones")
            mask = pp.tile([128, 512], F32R, tag="mask")

            nc.vector.memset(ones[:].bitcast(F32), 1.0)
            nc.sync.dma_start(mask[:], mk_d.ap())
            nc.vector.memset(V[:, :, :, 64:66].bitcast(F32), 1.0)   # fused ones columns
            nc.vector.memset(V[:, :, :, 66:129].bitcast(F32), 0.0)  # zero pad in odd blocks

            # ---- Phase A: QKV projections ----
            x_sb = xw.tile([128, KCQ, T], F32R)
            wq_sb = xw.tile([128, KCQ, JD], F32R)
            wk_sb = xw.tile([128, KCQ, JD], F32R)
            wv_sb = xw.tile([128, KCQ, JD], F32R)
            for k in range(KCQ):
                kp = min(128, KQ - 128 * k)
                nc.sync.dma_start(x_sb[0:kp, k, :], xT_d.ap()[128 * k:128 * k + kp, :])
                nc.sync.dma_start(wq_sb[0:kp, k, :], wq_d.ap()[128 * k:128 * k + kp, :])
                nc.sync.dma_start(wk_sb[0:kp, k, :], wk_d.ap()[128 * k:128 * k + kp, :])
                nc.sync.dma_start(wv_sb[0:kp, k, :], wv_d.ap()[128 * k:128 * k + kp, :])

            # qT / kT: out (128 j x 512 t) tiles, K accumulated over chunks
            for w_sb, dest in ((wq_sb, qT), (wk_sb, kT)):
                for jt in range(2):
                    pss = [psA.tile([128, 512], F32, tag=f"a{t}", name=f"pss{t}") for t in range(4)]
                    for k in range(KCQ):
                        kp = min(128, KQ - 128 * k)
                        for t4 in range(4):
                            nc.tensor.matmul(
                                pss[t4][:],
                                w_sb[0:kp, k, jt * 128:(jt + 1) * 128],
                                x_sb[0:kp, k, t4 * 512:(t4 + 1) * 512],
                                start=(k == 0), stop=(k == KCQ - 1),
                            )
                    for t4 in range(4):
                        nc.scalar.copy(dest[jt][:, t4 * 512:(t4 + 1) * 512], pss[t4][:])

            # v: out (128 t x 256 d) tiles, split per head into V with ones cols
            for tt in range(TT):
                vps = psA.tile([128, JD], F32, tag=f"a{tt % 4}")
                for k in range(KCQ):
                    kp = min(128, KQ - 128 * k)
                    nc.tensor.matmul(
                        vps[:],
                        x_sb[0:kp, k, tt * 128:(tt + 1) * 128],
                        wv_sb[0:kp, k, :],
                        start=(k == 0), stop=(k == KCQ - 1),
                    )
                for h in range(NHL):
                    jt, half = h // 2, h % 2
                    off = 0 if half == 0 else 129
                    nc.vector.tensor_copy(
                        V[:, tt, jt, off:off + 64], vps[:, h * 64:(h + 1) * 64])

        # ---- Phase B: causal attention ----
        with (
            tc.tile_pool(name="ptp", bufs=3) as ptp,
            tc.tile_pool(name="rcp", bufs=2) as rcp,
            tc.tile_pool(name="psS", bufs=2, space="PSUM") as psS,
            tc.tile_pool(name="psY", bufs=1, space="PSUM") as psY,
            tc.tile_pool(name="psB", bufs=1, space="PSUM") as psB,
        ):
            for jt in range(2):
                for j4 in range(TC4):
                    ypE = psY.tile([65, 512], F32, tag="ypE")
                    ypO = psY.tile([128, 512], F32, tag="ypO")
                    ni = 4 * j4 + 4
                    for i in range(ni):
                        r = i - 4 * j4
                        w = 512 if r < 0 else 512 - 128 * r
                        co = 512 - w
                        for half in range(2):
                            p0, p1 = half * 64, (half + 1) * 64
                            sp = psS.tile([128, 512], F32, tag=f"s{half}")
                            nc.tensor.matmul(
                                sp[:, 0:w],
                                kT[jt][p0:p1, i * 128:(i + 1) * 128],
                                qT[jt][p0:p1, j4 * 512 + co:(j4 + 1) * 512],
                                start=True, stop=True,
                            )
                            pt = ptp.tile([128, 512], F32R, tag=f"p{half}")
                            nc.scalar.activation(pt[:, 0:w], sp[:, 0:w], EXP, scale=SCALE)
                            if r >= 0:
                                nc.vector.tensor_mul(pt[:, 0:w], pt[:, 0:w], mask[:, 0:w])
                            yp = ypE if half == 0 else ypO
                            vs = V[:, i, jt, 0:65] if half == 0 else V[:, i, jt, 65:193]
                            nc.tensor.matmul(
                                yp[:, co:512], vs, pt[:, 0:w],
                                start=(i == 0), stop=(i == ni - 1),
                            )
                    # normalize: yTn = y * bcast(1/denom)
                    ts4 = slice(j4 * 512, (j4 + 1) * 512)
                    recE = rcp.tile([128, 512], F32R, tag="recE")
                    recO = rcp.tile([128, 512], F32R, tag="recO")
                    with nc.allow_low_precision(reason="float32r is fp32 storage"):
                        nc.vector.reciprocal(recE[64:65, :], ypE[64:65, :])
                        nc.vector.reciprocal(recO[0:1, :], ypO[0:1, :])
                    bcE = psB.tile([64, 512], F32, tag="bcE")
                    nc.tensor.matmul(bcE[:], ones[64:65, 0:64], recE[64:65, :],
                                     start=True, stop=True)
                    bcO = psB.tile([128, 512], F32, tag="bcO")
                    nc.tensor.matmul(bcO[:], ones[0:1, 0:128], recO[0:1, :],
                                     start=True, stop=True)
                    yE_sb = rcp.tile([64, 512], F32R, tag="yE")
                    nc.scalar.copy(yE_sb[:], ypE[0:64, :])
                    nc.vector.tensor_mul(yTn[jt][0:64, ts4], yE_sb[:], bcE[:])
                    yO_sb = rcp.tile([128, 512], F32R, tag="yO")
                    nc.scalar.copy(yO_sb[64:128, :], ypO[64:128, :])
                    nc.vector.tensor_mul(yTn[jt][64:128, ts4], yO_sb[64:128, :],
                                         bcO[64:128, :])

        # ---- Phase C: output projection (partial: our 256 y-dims) ----
        with (
            tc.tile_pool(name="wpp", bufs=1) as wpp,
            tc.tile_pool(name="outp", bufs=3) as outp,
            tc.tile_pool(name="psC", bufs=2, space="PSUM") as psC,
        ):
            wp_sb = wpp.tile([128, 2, C], F32R)
            for kc in range(2):
                nc.sync.dma_start(wp_sb[:, kc, :], wp_d.ap()[128 * kc:128 * (kc + 1), :])
            for tt in range(TT):
                ops = psC.tile([128, C], F32, tag="o")
                for nh in range(2):
                    for kc in range(2):
                        nc.tensor.matmul(
                            ops[:, nh * 512:(nh + 1) * 512],
                            yTn[kc][:, tt * 128:(tt + 1) * 128],
                            wp_sb[:, kc, nh * 512:(nh + 1) * 512],
                            start=(kc == 0), stop=(kc == 1),
                        )
                ob = outp.tile([128, C], F32, tag="ob")
                nc.vector.tensor_copy(ob[:], ops[:])
                nc.sync.dma_start(out_d.ap()[tt * 128:(tt + 1) * 128, :], ob[:])

    nc.compile()
    return nc


_PROGRAM_CACHE = {}


def get_program(qkv_bias: bool):
    if qkv_bias not in _PROGRAM_CACHE:
        _PROGRAM_CACHE[qkv_bias] = build_program(qkv_bias)
    return _PROGRAM_CACHE[qkv_bias]


def make_in_maps(x, W_qkv, b_qkv, W_proj):
    qkv_bias = bool(np.any(b_qkv != 0.0))
    mk = (np.arange(128)[:, None] <= np.arange(512)[None, :]).astype(np.float32)
    in_maps = []
    for c in range(N_CORES):
        b, hg = c // 4, c % 4
        r0 = hg * JD                       # first q/k/v row for this head group
        xT = np.ascontiguousarray(x[b].T)
        wq = np.ascontiguousarray(W_qkv[r0:r0 + JD, :].T)
        wk = np.ascontiguousarray(W_qkv[C + r0:C + r0 + JD, :].T)
        wv = np.ascontiguousarray(W_qkv[2 * C + r0:2 * C + r0 + JD, :].T)
        if qkv_bias:
            xT = np.concatenate([xT, np.ones((1, T), np.float32)], axis=0)
            wq = np.concatenate([wq, b_qkv[None, r0:r0 + JD]], axis=0)
            wk = np.concatenate([wk, b_qkv[None, C + r0:C + r0 + JD]], axis=0)
            wv = np.concatenate([wv, b_qkv[None, 2 * C + r0:2 * C + r0 + JD]], axis=0)
        wp = np.ascontiguousarray(W_proj[:, r0:r0 + JD].T)
        in_maps.append({
            "xT": np.ascontiguousarray(xT),
            "wq": np.ascontiguousarray(wq),
            "wk": np.ascontiguousarray(wk),
            "wv": np.ascontiguousarray(wv),
            "wp": wp,
            "mk": mk,
        })
    return in_maps, qkv_bias


def kernel(x, W_qkv, b_qkv, W_proj, b_proj):
    x = np.asarray(x, dtype=np.float32)
    W_qkv = np.asarray(W_qkv, dtype=np.float32)
    b_qkv = np.asarray(b_qkv, dtype=np.float32)
    W_proj = np.asarray(W_proj, dtype=np.float32)
    b_proj = np.asarray(b_proj, dtype=np.float32)

    in_maps, qkv_bias = make_in_maps(x, W_qkv, b_qkv, W_proj)
    nc = get_program(qkv_bias)
    results = run_bass_kernel_spmd(nc, in_maps, core_ids=list(range(N_CORES))).results

    out = np.empty((B, T, C), dtype=np.float32)
    for b in range(B):
        acc = results[4 * b]["out"].copy()
        for hg in range(1, 4):
            acc += results[4 * b + hg]["out"]
        out[b] = acc + b_proj[None, :]
    return out
